# revision 38
# baseline (speedup 1.0000x reference)
"""Trainium2 Bass kernel for the sparse_attention nn_Kernel problem.

Math (per sample, derived from the reference):
  t1 = p1w * x ; t2 = roll(t1, 1, ch) ; t3_k = shift_{k-3}(t2) (zero-padded, w)
  C_k[i,m] = sum_p x[i,p] * t1pad[m, p+s]   (s = k-3; c x c)
  t7m_k[q=m, i] = A[q] - C_k^T[q]  where A[q] = C_3^T[q+1] (partition roll)
  Sm[q=m] = 7*A[q] - sum_k C_k^T[q] ; Sj[q] = Sm[q-1]
  out = roll_{h+1,w-1}( sum_k t7m_k^T @ t1pad(col shift s) + Sj^T @ invroll_hw(x) )

Key layout/schedule tricks vs the original baseline:
  - the 7 shifted transposed operands t3_kT[p',m] = t1pad[m, p'+s] are built by
    XBAR DMA transposes reading the t1 buffer at a free-dim column offset s
    (slack zeros give exact unfold semantics), NOT via a DRAM partition-shift
    bounce.  This halves DMA traffic and removes ~120us of serialized queue
    time.  All XBAR transposes stay on ONE queue (sync): concurrent XBAR
    transposes on both hwdge queues corrupt each other (shared XBAR unit).
  - plain DRAM loads/stores and small partition-roll copies go on the OTHER
    hwdge queue (scalar) and overlap the XBAR phase safely.
  - bmm2's ib=0 accumulation is interleaved into the bmm1 k-loop (7 PSUM
    Q-tiles stay open across k), so the XBAR-transpose-bound phase also
    retires bmm2 work; ib=1 runs as a second pass from the persistent t7
    tiles.
  - pad-only memsets, issued before the loads; element-wise work split
    across vector / scalar / gpsimd; final cyclic roll folded into the
    PSUM->SBUF copies.

Each of the 8 cores processes one sample of the batch (data parallel).
"""

import math

import numpy as np

C = 256
H = 56
W = 56
WP = 64  # padded width
PADL = 3
NPP = H * WP  # 3584 padded positions
NCH = NPP // 128  # 28 chunks of 128 partitions
K = 7
SL = 8  # slack zero cols each side of t1buf so shifted transposes stay in range
BETA = 1.0 / (math.sqrt(H * W) * math.sqrt(C * K))
N_CORES = 8
HT = 8  # h rows per bmm2 out tile
NQT = H // HT  # 7 tiles per ib

_CACHE = {}


def _build_nc():
    import concourse.mybir as mybir
    import concourse.tile as tile
    from concourse import bacc

    f32 = mybir.dt.float32
    bf16 = mybir.dt.bfloat16

    nc = bacc.Bacc("TRN2", target_bir_lowering=False, debug=False)

    xin = nc.dram_tensor("x", [C, H, W], f32, kind="ExternalInput").ap()
    pwin = nc.dram_tensor("p1w", [C, H, W], f32, kind="ExternalInput").ap()
    out = nc.dram_tensor("out", [C, H, W], f32, kind="ExternalOutput").ap()

    sub = mybir.AluOpType.subtract
    mult = mybir.AluOpType.mult
    add = mybir.AluOpType.add

    with tile.TileContext(nc) as tc:
        with (
            tc.tile_pool(name="f32big", bufs=1) as pf32,
            tc.tile_pool(name="bfbig", bufs=1) as pbf,
            tc.tile_pool(name="bfroll", bufs=1) as pbr,
            tc.tile_pool(name="ptrans", bufs=1) as pxT,
            tc.tile_pool(name="pt3", bufs=3) as pt3,
            tc.tile_pool(name="small", bufs=1) as psm,
            tc.tile_pool(name="ps1", bufs=2, space="PSUM") as pps1,
            tc.tile_pool(name="ps2", bufs=4, space="PSUM") as pps2,
            tc.tile_pool(name="pstr", bufs=2, space="PSUM") as ppstr,
        ):
            # ------------- tiles + pad memsets (before loads) -------------
            x_cp, p_cp, x_bf, t1buf = [], [], [], []
            for cb in range(2):
                xt = pf32.tile([128, H, W], f32, tag=f"xcp{cb}")
                x_cp.append(xt)
                pt = pf32.tile([128, H, W], f32, tag=f"pcp{cb}")
                p_cp.append(pt)

                tb = pbf.tile([128, 2 * SL + NPP], bf16, tag=f"t1b{cb}")
                nc.vector.memset(tb[:, 0:SL], 0.0)
                nc.vector.memset(tb[:, SL + NPP : 2 * SL + NPP], 0.0)
                tb3 = tb[:, SL : SL + NPP].rearrange("p (h w) -> p h w", w=WP)
                nc.vector.memset(tb3[:, :, 0:PADL], 0.0)
                nc.vector.memset(tb3[:, :, PADL + W : WP], 0.0)
                t1buf.append(tb)

                xb = pbf.tile([128, NPP], bf16, tag=f"xbf{cb}")
                xb3 = xb.rearrange("p (h w) -> p h w", w=WP)
                nc.vector.memset(xb3[:, :, 0:PADL], 0.0)
                nc.vector.memset(xb3[:, :, PADL + W : WP], 0.0)
                x_bf.append(xb)

            # ------------- loads: h-halves, x on sync / p1w on scalar ------
            # (p tile is shared across cb: its cb-loads are emitted inside
            # the prep loop, after the previous cb's mul reads)
            HH = H // 2
            for cb in range(2):
                for hb in range(2):
                    hs = slice(hb * HH, (hb + 1) * HH)
                    nc.sync.dma_start(
                        x_cp[cb][:, hs, :],
                        xin[cb * 128 : (cb + 1) * 128][:, hs, :],
                    )
                    nc.scalar.dma_start(
                        p_cp[cb][:, hs, :],
                        pwin[cb * 128 : (cb + 1) * 128][:, hs, :],
                    )

            # ------------- padded bf16 operands (h-half pieces) ------------
            for cb in range(2):
                tb3 = t1buf[cb][:, SL : SL + NPP].rearrange(
                    "p (h w) -> p h w", w=WP
                )
                xb3 = x_bf[cb].rearrange("p (h w) -> p h w", w=WP)
                for hb in range(2):
                    hs = slice(hb * HH, (hb + 1) * HH)
                    nc.vector.tensor_mul(
                        tb3[:, hs, PADL : PADL + W],
                        x_cp[cb][:, hs, :],
                        p_cp[cb][:, hs, :],
                    )
                    nc.vector.tensor_scalar_mul(
                        xb3[:, hs, PADL : PADL + W], x_cp[cb][:, hs, :], BETA
                    )

            def t1view(cb):
                return t1buf[cb][:, SL : SL + NPP].rearrange(
                    "p (h w) -> p h w", w=WP
                )

            # ------------- transposes --------------------------------------
            # chunks [0,14) ('a', rows 0..28) go through the PE array (tensor
            # + vector are idle during the front); chunks [14,28) ('b') and
    	    # all t3k tensors go through the XBAR on sync.
            from concourse.masks import make_identity

            ident = psm.tile([128, 128], bf16, tag="ident")
            make_identity(nc, ident[:])

            t1T = pxT.tile([128, NCH, C], bf16, tag="t1T")
            xpT = pxT.tile([128, NCH, C], bf16, tag="xpT")
            HNP = NPP // 2

            def pe_transpose_piece(dst, srcbuf, src_off, cb):
                # chunks 0..13 of dst's cb-half from srcbuf via PE
                for t in range(14):
                    tr = ppstr.tile([128, 128], bf16, tag="pstr")
                    nc.tensor.transpose(
                        tr[:],
                        srcbuf[:, src_off + 128 * t : src_off + 128 * (t + 1)],
                        ident[:],
                    )
                    nc.vector.tensor_copy(
                        out=dst[:, t, cb * 128 : (cb + 1) * 128], in_=tr[:]
                    )

            for cb in range(2):
                pe_transpose_piece(t1T, t1buf[cb], SL, cb)
                pe_transpose_piece(xpT, x_bf[cb], 0, cb)
                cs = slice(14, 28)
                nc.sync.dma_start_transpose(
                    t1T[:, cs, cb * 128 : (cb + 1) * 128],
                    t1buf[cb][:, SL + HNP : SL + NPP],
                )
                nc.sync.dma_start_transpose(
                    xpT[:, cs, cb * 128 : (cb + 1) * 128],
                    x_bf[cb][:, HNP:NPP],
                )

            def make_t3T(k):
                s = k - 3
                t3k = pt3.tile([128, NCH, C], bf16, tag="t3")
                for cb in range(2):
                    nc.sync.dma_start_transpose(
                        t3k[:, :, cb * 128 : (cb + 1) * 128],
                        t1buf[cb][:, SL + s : SL + s + NPP],
                    )
                return t3k

            # xroll[j, h', w'] = x[j, (h'+1)%H, (w'-1)%W]  (bf16, for S-term;
            # vector, after the transpose-critical prep)
            xroll = []
            for cb in range(2):
                xr = pbr.tile([128, H, W], bf16, tag=f"xroll{cb}")
                nc.vector.tensor_copy(
                    out=xr[:, 0:55, 1:W], in_=x_cp[cb][:, 1:56, 0 : W - 1]
                )
                nc.vector.tensor_copy(
                    out=xr[:, 0:55, 0:1], in_=x_cp[cb][:, 1:56, W - 1 : W]
                )
                nc.vector.tensor_copy(
                    out=xr[:, 55:56, 1:W], in_=x_cp[cb][:, 0:1, 0 : W - 1]
                )
                nc.vector.tensor_copy(
                    out=xr[:, 55:56, 0:1], in_=x_cp[cb][:, 0:1, W - 1 : W]
                )
                xroll.append(xr)

            # ------------- bmm1 set helper ---------------------------------
            def bmm1_set(Tw):
                tiles = []
                for mb in range(2):
                    pt = pps1.tile([128, C], f32, tag="ps1")
                    for t in range(NCH):
                        nc.tensor.matmul(
                            pt[:],
                            Tw[:, t, mb * 128 : mb * 128 + 128],
                            xpT[:, t, :],
                            start=(t == 0),
                            stop=(t == NCH - 1),
                        )
                    tiles.append(pt)
                return tiles

            # ------------- bmm2 ib=0 Q tiles (persist across the k loop) ---
            NQ0 = 4  # tiles fused into the k loop (PSUM bank budget)
            q_tiles0 = []
            for j in range(NQ0):
                q = pps2.tile([128, HT * W], f32, tag="ps2")
                q_tiles0.append(q.rearrange("p (h w) -> p h w", w=W))

            h0s0 = [j * HT for j in range(NQ0)]
            t7 = {}

            def bmm2_partial(k, qviews, h0s, ib, first, last=False):
                s = k - 3
                for mb in range(2):
                    for qi, (qv, h0) in enumerate(zip(qviews, h0s)):
                        rhs = t1view(mb)[
                            :, h0 : h0 + HT, PADL + s : PADL + s + W
                        ]
                        nc.tensor.matmul(
                            qv[:],
                            t7[(k, mb)][:, ib * 128 : (ib + 1) * 128],
                            rhs,
                            start=(first and mb == 0),
                            stop=(last and mb == 1),
                        )

            def bmm2_sterm(qviews, h0s, ib):
                # S-term plus the A-term of G (A^T @ box7(t1))
                for mb in range(2):
                    for qv, h0 in zip(qviews, h0s):
                        nc.tensor.matmul(
                            qv[:],
                            Sj_bf[mb][:, ib * 128 : (ib + 1) * 128],
                            xroll[mb][:, h0 : h0 + HT, :],
                            start=(mb == 0),
                            stop=False,
                        )
                for mb in range(2):
                    for qv, h0 in zip(qviews, h0s):
                        nc.tensor.matmul(
                            qv[:],
                            A_bf[mb][:, ib * 128 : (ib + 1) * 128],
                            Bbox[mb][:, h0 : h0 + HT, :],
                            start=False,
                            stop=(mb == 1),
                        )

            # ------------- C_3, A roll, then k loop with fused bmm2 --------
            C3_ps = bmm1_set(t1T)
            C3_sb = []
            ckbf = {}
            for mb in range(2):
                c3 = psm.tile([128, C], bf16, tag=f"c3sb{mb}")
                nc.vector.tensor_copy(out=c3[:], in_=C3_ps[mb][:])
                C3_sb.append(c3)
                cn = psm.tile([128, C], bf16, tag=f"ck_3_{mb}")
                nc.vector.tensor_scalar_mul(cn[:], C3_ps[mb][:], -1.0)
                t7[(3, mb)] = cn
            A_bf = []
            for mb in range(2):
                asb = psm.tile([128, C], bf16, tag=f"abf{mb}")
                A_bf.append(asb)

            def emit_a_rolls():
                # partition-roll of C_3 (small direct2d copies on sync,
                # emitted late so they never gate the transpose stream)
                for mb in range(2):
                    nc.sync.dma_start(A_bf[mb][0:127, :], C3_sb[mb][1:128, :])
                    nc.sync.dma_start(
                        A_bf[mb][127:128, :], C3_sb[1 - mb][0:1, :]
                    )

            # Wneg[mb] accumulates sum_k (-C_k^T) (bf16)
            W_acc = []
            for mb in range(2):
                wa = psm.tile([128, C], bf16, tag=f"wacc{mb}")
                nc.vector.tensor_scalar_mul(wa[:], C3_ps[mb][:], -1.0)
                W_acc.append(wa)

            # k loop: drain each C_k PSUM tile as a NEGATED bf16 copy (no A
            # dependency anywhere); the A-term of G is folded into the late
            # S-term pass via G = A^T @ box7(t1) - sum_k C_k^T @ rhs_k.
            korder = (0, 1, 2, 4, 5, 6)
            bmm2_done = []
            for idx, k in enumerate(korder):
                t3T = make_t3T(k)
                ck = bmm1_set(t3T)
                for mb in range(2):
                    cn = psm.tile([128, C], bf16, tag=f"ck_{k}_{mb}")
                    nc.vector.tensor_scalar_mul(cn[:], ck[mb][:], -1.0)
                    t7[(k, mb)] = cn
                    nc.vector.tensor_tensor(
                        W_acc[mb][:], W_acc[mb][:], cn[:], add
                    )
                if idx == 1:
                    bmm2_partial(3, q_tiles0, h0s0, 0, first=True)
                    bmm2_done.append(3)
                elif idx >= 2:
                    bmm2_partial(korder[idx - 2], q_tiles0, h0s0, 0, first=False)
                    bmm2_done.append(korder[idx - 2])
            for kk in (korder[-2], korder[-1]):
                bmm2_partial(
                    kk, q_tiles0, h0s0, 0, first=False, last=(kk == korder[-1])
                )
                bmm2_done.append(kk)
            assert sorted(bmm2_done) == sorted(range(K))

            emit_a_rolls()

            # B = box7(t1pad) (bf16) and A_bf for the late A-term matmuls
            Bbox = []
            for mb in range(2):
                bb = pbr.tile([128, H, W], bf16, tag=f"bbox{mb}")
                nc.vector.tensor_tensor(
                    bb[:],
                    t1view(mb)[:, :, 0:W],
                    t1view(mb)[:, :, 1 : 1 + W],
                    add,
                )
                for s in range(2, 7):
                    nc.vector.tensor_tensor(
                        bb[:], bb[:], t1view(mb)[:, :, s : s + W], add
                    )
                Bbox.append(bb)
            # Sm / Sj  (Sm computed in place into the W accumulator)
            Sm_bf = W_acc
            for mb in range(2):
                nc.vector.scalar_tensor_tensor(
                    W_acc[mb][:], A_bf[mb][:], 7.0, W_acc[mb][:], mult, add
                )
            Sj_bf = []
            for mb in range(2):
                sj = psm.tile([128, C], bf16, tag=f"sjbf{mb}")
                Sj_bf.append(sj)
            for mb in range(2):
                nc.sync.dma_start(Sj_bf[mb][1:128, :], Sm_bf[mb][0:127, :])
                nc.sync.dma_start(Sj_bf[mb][0:1, :], Sm_bf[1 - mb][127:128, :])

            # ------------- finish ib=0, run ib=1, roll + store -------------
            out_sb = []
            for ib in range(2):
                osb = pf32.tile([128, H, W], f32, tag=f"osb{ib}")
                out_sb.append(osb)

            def roll_and_copy(qviews, h0s, ib, accum=False, jofs=0):
                for j, (qv, h0) in enumerate(zip(qviews, h0s)):
                    use_scalar_copy = (j + jofs) % 2 == 1 and not accum

                    def cp(o, i):
                        if accum:
                            nc.vector.tensor_tensor(o, o, i, add)
                        elif use_scalar_copy:
                            nc.scalar.copy(o, i)
                        else:
                            nc.vector.tensor_copy(out=o, in_=i)

                    def roll_copy(r0, r1, d0):
                        cp(
                            out_sb[ib][:, d0 : d0 + (r1 - r0), 0 : W - 1],
                            qv[:, r0:r1, 1:W],
                        )
                        cp(
                            out_sb[ib][:, d0 : d0 + (r1 - r0), W - 1 : W],
                            qv[:, r0:r1, 0:1],
                        )

                    if h0 + HT < H:
                        roll_copy(0, HT, h0 + 1)
                    else:
                        roll_copy(0, HT - 1, h0 + 1)
                        roll_copy(HT - 1, HT, 0)

            roll_and_copy(q_tiles0, h0s0, 0)

            # ------------- pass 2 + interleaved S-term batches -------------
            sterm_queue = [(0, j * HT) for j in range(NQ0)]  # q0 tiles first
            piece_ct = [0]

            def emit_piece(ib, h0, accum=False):
                ob = out[ib * 128 : (ib + 1) * 128]
                eng = nc.sync if piece_ct[0] % 2 == 0 else nc.scalar
                piece_ct[0] += 1
                if h0 + HT < H:
                    eng.dma_start(
                        ob[:, h0 + 1 : h0 + 1 + HT, :],
                        out_sb[ib][:, h0 + 1 : h0 + 1 + HT, :],
                    )
                else:
                    eng.dma_start(
                        ob[:, h0 + 1 : H, :], out_sb[ib][:, h0 + 1 : H, :]
                    )
                    eng.dma_start(ob[:, 0:1, :], out_sb[ib][:, 0:1, :])

            def emit_sterm_batch(batch):
                qvs, h0s_g, ibs_g = [], [], []
                for ib, h0 in batch:
                    q = pps2.tile([128, HT * W], f32, tag="ps2")
                    qvs.append(q.rearrange("p (h w) -> p h w", w=W))
                    h0s_g.append(h0)
                    ibs_g.append(ib)
                for mb in range(2):
                    for qv, h0, ib in zip(qvs, h0s_g, ibs_g):
                        nc.tensor.matmul(
                            qv[:],
                            Sj_bf[mb][:, ib * 128 : (ib + 1) * 128],
                            xroll[mb][:, h0 : h0 + HT, :],
                            start=(mb == 0),
                            stop=False,
                        )
                for mb in range(2):
                    for qv, h0, ib in zip(qvs, h0s_g, ibs_g):
                        nc.tensor.matmul(
                            qv[:],
                            A_bf[mb][:, ib * 128 : (ib + 1) * 128],
                            Bbox[mb][:, h0 : h0 + HT, :],
                            start=False,
                            stop=(mb == 1),
                        )
                for qv, h0, ib in zip(qvs, h0s_g, ibs_g):
                    roll_and_copy([qv], [h0], ib, accum=True)
                    emit_piece(ib, h0, accum=False)

            items = [(0, j * HT) for j in range(NQ0, NQT)] + [
                (1, j * HT) for j in range(NQT)
            ]
            n_done = 0
            for g0 in range(0, len(items), 2):
                grp = items[g0 : g0 + 2]
                qvs, h0s_g, ibs_g = [], [], []
                for ib, h0 in grp:
                    q = pps2.tile([128, HT * W], f32, tag="ps2")
                    qvs.append(q.rearrange("p (h w) -> p h w", w=W))
                    h0s_g.append(h0)
                    ibs_g.append(ib)
                ks = (3, 0, 1, 2, 4, 5, 6)
                for idx, k in enumerate(ks):
                    s = k - 3
                    for mb in range(2):
                        for qv, h0, ib in zip(qvs, h0s_g, ibs_g):
                            rhs = t1view(mb)[
                                :, h0 : h0 + HT, PADL + s : PADL + s + W
                            ]
                            nc.tensor.matmul(
                                qv[:],
                                t7[(k, mb)][:, ib * 128 : (ib + 1) * 128],
                                rhs,
                                start=(idx == 0 and mb == 0),
                                stop=(idx == len(ks) - 1 and mb == 1),
                            )
                for qv, h0, ib in zip(qvs, h0s_g, ibs_g):
                    roll_and_copy([qv], [h0], ib, jofs=n_done)
                    n_done += 1
                # interleave an S-term batch for already-finalized tiles
                if sterm_queue:
                    batch, sterm_queue = sterm_queue[:2], sterm_queue[2:]
                    emit_sterm_batch(batch)
                sterm_queue = sterm_queue + [(ib, h0) for ib, h0 in grp]

            # ------------- drain remaining S-term batches ------------------
            while sterm_queue:
                batch, sterm_queue = sterm_queue[:2], sterm_queue[2:]
                emit_sterm_batch(batch)

    nc.compile()
    return nc


def _get_nc():
    if "nc" not in _CACHE:
        _CACHE["nc"] = _build_nc()
    return _CACHE["nc"]


def kernel(x: np.ndarray, p1w: np.ndarray) -> np.ndarray:
    from concourse.bass_utils import run_bass_kernel_spmd

    n = x.shape[0]
    assert n == N_CORES
    x = np.ascontiguousarray(np.asarray(x, dtype=np.float32))
    pw = np.ascontiguousarray(np.asarray(p1w, dtype=np.float32)[0])

    nc = _get_nc()
    in_maps = [{"x": x[i], "p1w": pw} for i in range(n)]
    res = run_bass_kernel_spmd(nc, in_maps, list(range(N_CORES)))
    outs = [res.results[i]["out"] for i in range(n)]
    return np.stack(outs, axis=0).astype(np.float32)
